# revision 18
# baseline (speedup 1.0000x reference)
"""Causal self-attention (B=4, T=2048, D=1024, H=16) on 8 Trainium2 cores.

Sharding: core m = (batch b=m//2, head-group g=m%2 of 8 heads) — data parallel
over batch, tensor parallel over heads.  No device collectives at all: the
projection is row-parallel (each core multiplies its own 512 y-channels by its
512-row slice of w_proj over all 2048 tokens) and returns a PARTIAL [2048,
1024] f32 output; the host sums each core pair's partials while unsharding.

Precision: all matmul operands are bf16 (x, w, q, k, v, exp(S), y); psum
accumulation f32.  fp8 was tried and rejected: quantization noise in a dot
product of independent terms does not average down (signal and noise both
grow as sqrt(N)), so ANY fp8 operand costs 2-4% output RMS vs the 2e-2 gate.
Measured rel err: ~4e-3.

Schedule (the point of this kernel): per 512-token chunk j, attention runs
as 8 (head-pair, 64-row band) groups.  exp(S/8) on the Act engine is the
second-largest engine load, so each group emits its score matmuls eagerly
(diagonal: a 2-bank ragged pack + one 256-wide tile; off-diagonal: 2-bank
kt pairs), and defers its last attn@V matmuls + normalize behind the NEXT
group's score matmuls (attn@V trails exp by 2 pairs).  PSUM score banks are
freed by the exp read itself, so 2 rotating pair-slots suffice.  Next-chunk
QKV production is interleaved one piece per group via hooks, and the
projection of chunk j overlaps attention of chunk j+1.  Causal masking
multiplies the diagonal 128x128 blocks by a lower-tri mask on the Pool
engine after exp; a ones-column in V yields softmax denominators for free.
"""

import numpy as np

import concourse.bass as bass
import concourse.mybir as mybir
import concourse.tile as tile
from concourse import bacc
from concourse.bass_utils import run_bass_kernel_spmd

F32 = mybir.dt.float32
BF = mybir.dt.bfloat16
F8 = mybir.dt.float8e4

# Problem constants (per spec; hardcoded).
B, T, D, H = 4, 2048, 1024, 16
DH = 64                      # head dim
N_CORES = 8
HC = H // 2                  # heads per core = 8
HP = HC // 2                 # head pairs per core = 4
DK = D // 128                # model-dim contraction tiles = 8
KP = DK // 2                 # DoubleRow contraction pairs = 4
TT = T // 128                # token tiles of 128 = 16
TC = T // 512                # token chunks of 512 = 4
SCALE = 1.0 / 8.0            # 1/sqrt(DH)
EBIAS = -1.0                 # exp bias, guards fp8e4m3 overflow/underflow
WS = 32.0                    # host-side weight scale (w ~ N(0,1/32^2))
NO_COLLECTIVE = False        # kept for tooling compat; kernel has none
DEBUG = False


def build_kernel(iters=1):
    nc = bacc.Bacc("TRN2", target_bir_lowering=False, debug=False,
                   num_devices=N_CORES)

    x_bT = nc.dram_tensor("x_bT", [D, T], BF, kind="ExternalInput").ap()
    w_qkv_my = nc.dram_tensor("w_qkv_my", [D, 3 * 512], BF,
                              kind="ExternalInput").ap()
    w_proj_my = nc.dram_tensor("w_proj_my", [512, D], BF,
                               kind="ExternalInput").ap()
    out = nc.dram_tensor("out", [T, D], F32, kind="ExternalOutput").ap()

    with tile.TileContext(nc) as tc:
        for _ in range(iters):
            _emit(tc, x_bT, w_qkv_my, w_proj_my, out)

    nc.compile()
    return nc


def _emit(tc, x_bT, w_qkv_my, w_proj_my, out):
    from contextlib import ExitStack
    nc = tc.nc
    ctx = ExitStack()

    # ---- constants ------------------------------------------------------
    const = ctx.enter_context(tc.tile_pool(name="const", bufs=1))
    mask_f32 = const.tile([128, 128], F32)
    nc.gpsimd.memset(mask_f32[:], 1.0)
    nc.gpsimd.affine_select(
        out=mask_f32[:], in_=mask_f32[:],
        compare_op=mybir.AluOpType.is_ge,
        fill=0.0, base=0,
        pattern=[[1, 128]],       # + qq
        channel_multiplier=-1,    # - kk
    )
    mask = const.tile([128, 128], BF)
    nc.gpsimd.tensor_copy(mask[:], mask_f32[:])

    # ---- persistent SBUF ------------------------------------------------
    persist = ctx.enter_context(tc.tile_pool(name="persist", bufs=1))
    wq_sb = persist.tile([128, DK, 512], BF, tag="wq")
    wk_sb = persist.tile([128, DK, 512], BF, tag="wk")
    wv_sb = persist.tile([128, DK, 512], BF, tag="wv")
    wp_sb = persist.tile([128, HP, D], BF, tag="wp")
    kt_sb = persist.tile([128, HP, T], BF, tag="kt")
    v_sb = persist.tile([128, TT, HC, 65], BF, tag="v")

    x_re = x_bT.rearrange("(o p) t -> p o t", p=128)
    w_re = w_qkv_my.rearrange("(o p) f -> p o f", p=128)
    # DMA order tracks first use: q/k weights + x chunk 0 gate the first
    # scores; v weights gate the first attn@V; w_proj only the first proj.
    nc.sync.dma_start(wq_sb[:], w_re[:, :, 0:512])

    # v pad cols [65:96] must be zero (DR garbage rows stay finite+unread),
    # col 64 = 1.0 supplies softmax denominators.
    nc.gpsimd.memset(v_sb[:, :, :, 64:65], 1.0)

    with tc.tile_pool(name="xt", bufs=2) as xtp, \
         tc.tile_pool(name="qt", bufs=2) as qtp, \
         tc.tile_pool(name="yb", bufs=2) as ybp, \
         tc.tile_pool(name="ep", bufs=3) as epp, \
         tc.tile_pool(name="ed", bufs=2) as edp, \
         tc.tile_pool(name="nrm", bufs=2) as nrmp, \
         tc.tile_pool(name="ost", bufs=2) as ostp, \
         tc.tile_pool(name="ps", bufs=1, space="PSUM") as psp:

        pending = []             # deferred closures (last O + norm per group)

        def flush():
            while pending:
                pending.pop(0)()

        def qkv_chunk_dma(jn):
            xt = xtp.tile([128, DK, 512], BF, tag="xt", name="xt")
            nc.sync.dma_start(xt[:], x_re[:, :, jn * 512:(jn + 1) * 512])
            qt = qtp.tile([128, HP, 512], BF, tag="qt", name="qt")
            return xt, qt

        def qk_piece(jn, xt, qt, p):
            # one of 8 per-chunk QK productions: p = 2*hp' + (0:q, 1:k)
            hpn, which = p // 2, p % 2
            wsb = wq_sb if which == 0 else wk_sb
            dst = (qt[:, hpn, :] if which == 0
                   else kt_sb[:, hpn, jn * 512:(jn + 1) * 512])
            ps = psp.tile([128, 512], F32, tag="mm512", name="qkps", bufs=2)
            for kc in range(DK):
                nc.tensor.matmul(
                    ps[:],
                    lhsT=wsb[:, kc, hpn * 128:(hpn + 1) * 128],
                    rhs=xt[:, kc, :],
                    start=(kc == 0), stop=(kc == DK - 1))
            nc.vector.tensor_copy(dst, ps[:])

        def v_chunk(jn, xt):
            for tt in range(4):
                ps = psp.tile([128, 512], F32, tag="mm512", name="vps",
                              bufs=2)
                for kc in range(DK):
                    nc.tensor.matmul(
                        ps[:],
                        lhsT=xt[:, kc, tt * 128:(tt + 1) * 128],
                        rhs=wv_sb[:, kc, :],
                        start=(kc == 0), stop=(kc == DK - 1))
                nc.vector.tensor_copy(
                    v_sb[:, jn * 4 + tt, :, 0:64],
                    ps[:].rearrange("p (h d) -> p h d", h=HC))

        def attn_group(j, hp, i, qt, yb, hook=None):
            """One (head-pair, band) attention group, software-pipelined:
            emits its S work eagerly, defers its last O + normalize into
            `pending` so the next group's S matmuls slot in front of them.
            `hook` (interleaved non-Act work, e.g. next-chunk QK) is emitted
            after the S-phase so it never delays the next exp."""
            h = 2 * hp + i
            P = 2 * j
            o_ps = psp.tile([128, 512], F32, tag=f"ops{i}",
                            name=f"ops{i}", bufs=1)
            qv = qt[64 * i:64 * (i + 1), hp, :]
            kv = kt_sb[64 * i:64 * (i + 1), hp, :]

            # diagonal scores: dk0|dk1|dk3 packed into 2 banks, dk2 separate
            pack = psp.tile([128, 1024], F32, tag="pair", name="pack", bufs=2)
            s256 = psp.tile([128, 512], F32, tag="mm512", name="s256", bufs=2)
            diag = ((0, 0, slice(0, 512)), (1, 128, slice(512, 896)),
                    (3, 384, slice(896, 1024)))
            for dk, o, sl in diag:
                kt_i = 4 * j + dk
                nc.tensor.matmul(
                    pack[:, sl], lhsT=kv[:, kt_i * 128:(kt_i + 1) * 128],
                    rhs=qv[:, o:512], start=True, stop=True,
                    tile_position=(64 * i, 0))
            kt_i = 4 * j + 2
            nc.tensor.matmul(
                s256[:, 0:256], lhsT=kv[:, kt_i * 128:(kt_i + 1) * 128],
                rhs=qv[:, 256:512], start=True, stop=True,
                tile_position=(64 * i, 0))

            flush()          # previous group's deferred last-O + normalize
            if hook is not None:
                hook()

            e_pack = edp.tile([128, 1024], BF, tag="epk", name="e_pack")
            nc.scalar.activation(e_pack[:], pack[:],
                                 mybir.ActivationFunctionType.Exp,
                                 scale=SCALE)
            e_s = edp.tile([128, 256], BF, tag="es", name="e_s")
            nc.scalar.activation(e_s[:], s256[:, 0:256],
                                 mybir.ActivationFunctionType.Exp,
                                 scale=SCALE)
            for eap in (e_pack[:, 0:128], e_pack[:, 512:640],
                        e_pack[:, 896:1024], e_s[:, 0:128]):
                nc.gpsimd.tensor_tensor(eap, eap, mask[:],
                                        mybir.AluOpType.mult)
            for rhs, dk, o in ((e_pack[:, 0:512], 0, 0),
                               (e_pack[:, 512:896], 1, 128),
                               (e_pack[:, 896:1024], 3, 384),
                               (e_s[:, 0:256], 2, 256)):
                kt_i = 4 * j + dk
                nc.tensor.matmul(
                    o_ps[0:65, o:512], lhsT=v_sb[:, kt_i, h, 0:65],
                    rhs=rhs, start=(dk == 0), stop=(j == 0 and dk == 2),
                    skip_group_check=True)

            # off-diagonal kt pairs: S bf16 -> exp pair -> O DR fp8.
            # O(kp) is emitted after S/exp(kp+1); the last O is deferred.
            e_pairs = []
            for kp in range(P):
                spair = psp.tile([128, 2, 512], F32, tag="pair",
                                 name="spair", bufs=2)
                for u in range(2):
                    kt_i = 2 * kp + u
                    nc.tensor.matmul(
                        spair[:, u, :],
                        lhsT=kv[:, kt_i * 128:(kt_i + 1) * 128],
                        rhs=qv[:], start=True, stop=True,
                        tile_position=(64 * i, 0))
                e_pair = epp.tile([128, 2, 512], BF, tag="epr",
                                  name="e_pair")
                nc.scalar.activation(e_pair[:], spair[:],
                                     mybir.ActivationFunctionType.Exp,
                                     scale=SCALE)
                e_pairs.append(e_pair)
                if kp > 1:
                    emit_odr(j, h, o_ps, e_pairs[kp - 2], kp - 2)

            def tail():
                for kpt in range(max(P - 2, 0), P):
                    emit_odr(j, h, o_ps, e_pairs[kpt], kpt)
                # normalize: y = num * (1/den); y_sb = 4*y_true in fp8
                rec = nrmp.tile([1, 512], F32, tag="rec", name="rec")
                nc.vector.reciprocal(rec[:], o_ps[64:65, :])
                rec_b = nrmp.tile([64, 512], F32, tag="recb", name="rec_b")
                nc.gpsimd.partition_broadcast(rec_b[:], rec[:])
                nc.vector.tensor_tensor(
                    yb[64 * i:64 * (i + 1), hp, :],
                    o_ps[0:64, :], rec_b[:], mybir.AluOpType.mult)
            pending.append(tail)

        def emit_odr(j, h, o_ps, e_pair, kp):
            for u in range(2):
                nc.tensor.matmul(
                    o_ps[0:65, :], lhsT=v_sb[:, 2 * kp + u, h, 0:65],
                    rhs=e_pair[:, u, :], start=False,
                    stop=(kp == 2 * j - 1 and u == 1),
                    skip_group_check=True)

        def proj_block(j, yb):
            ost = ostp.tile([128, 4, D], F32, tag="ost", name="ost")
            for tt in range(4):
                for nn in range(2):
                    ps = psp.tile([128, 512], F32, tag="mm512", name="pps",
                                  bufs=2)
                    for hp2 in range(HP):
                        nc.tensor.matmul(
                            ps[:],
                            lhsT=yb[:, hp2, tt * 128:(tt + 1) * 128],
                            rhs=wp_sb[:, hp2, nn * 512:(nn + 1) * 512],
                            start=(hp2 == 0), stop=(hp2 == HP - 1))
                    nc.vector.tensor_copy(
                        ost[:, tt, nn * 512:(nn + 1) * 512], ps[:])
                nc.sync.dma_start(
                    out[j * 512 + tt * 128:j * 512 + (tt + 1) * 128, :],
                    ost[:, tt, :])

        # ---- chunk 0 bootstrap: only hp=0's q/k + V gate the first group --
        xt, qt = qkv_chunk_dma(0)
        nc.sync.dma_start(wk_sb[:], w_re[:, :, 512:1024])
        qk_piece(0, xt, qt, 0)
        qk_piece(0, xt, qt, 1)
        nc.sync.dma_start(wv_sb[:], w_re[:, :, 1024:1536])
        nc.sync.dma_start(wp_sb[:],
                          w_proj_my.rearrange("(o p) f -> p o f", p=128))
        v_chunk(0, xt)

        for j in range(TC):
            if j < TC - 1:
                xt_n, qt_n = qkv_chunk_dma(j + 1)
            yb = ybp.tile([128, HP, 512], BF, tag="yb", name="yb")
            for hp in range(HP):
                for i in range(2):
                    g_idx = 2 * hp + i
                    if j == 0:
                        # finish chunk 0's own QK one group ahead of use,
                        # plus chunk 1's piece like every other block
                        def hook(g=g_idx):
                            if g + 2 < 8:
                                qk_piece(0, xt, qt, g + 2)
                            qk_piece(1, xt_n, qt_n, g)
                    elif j < TC - 1:
                        hook = (lambda g=g_idx: qk_piece(j + 1, xt_n, qt_n, g))
                    else:
                        hook = None
                    attn_group(j, hp, i, qt, yb, hook=hook)
            if j < TC - 1:
                v_chunk(j + 1, xt_n)
                xt, qt = xt_n, qt_n
            flush()
            proj_block(j, yb)

    ctx.close()


_NC_CACHE = None
LAST_RESULT = None


def _prep_in_maps(x, w_qkv, w_proj):
    import ml_dtypes
    bf16 = ml_dtypes.bfloat16
    in_maps = []
    for m in range(N_CORES):
        b, g = m // 2, m % 2
        w_my = np.concatenate(
            [w_qkv[:, g * 512:(g + 1) * 512],
             w_qkv[:, 1024 + g * 512:1024 + (g + 1) * 512],
             w_qkv[:, 2048 + g * 512:2048 + (g + 1) * 512]], axis=1)
        in_maps.append({
            "x_bT": np.ascontiguousarray(x[b].T).astype(bf16),
            "w_qkv_my": np.ascontiguousarray(w_my).astype(bf16),
            "w_proj_my": np.ascontiguousarray(
                w_proj[g * 512:(g + 1) * 512, :]).astype(bf16),
        })
    return in_maps


def kernel(x, w_qkv, w_proj):
    global _NC_CACHE, LAST_RESULT
    x = np.asarray(x, dtype=np.float32)
    w_qkv = np.asarray(w_qkv, dtype=np.float32)
    w_proj = np.asarray(w_proj, dtype=np.float32)

    if _NC_CACHE is None:
        _NC_CACHE = build_kernel()
    nc = _NC_CACHE

    in_maps = _prep_in_maps(x, w_qkv, w_proj)
    res = run_bass_kernel_spmd(nc, in_maps, core_ids=list(range(N_CORES)))
    LAST_RESULT = res
    out = np.empty((B, T, D), dtype=np.float32)
    for b in range(B):
        out[b] = res.results[2 * b]["out"] + res.results[2 * b + 1]["out"]
    return out
